# revision 9
# baseline (speedup 1.0000x reference)
"""CRF loss (forward-algorithm partition function minus gold path score, batch mean)
as a Bass/Tile kernel on 8 Trainium2 NeuronCores.

Problem: nn_CRF_76647986364981
  feats [512, 1024, 48] f32, transitions [48, 48] f32,
  tags [1024, 512] int, lens [1024] int -> scalar f32 loss.

Strategy (data-parallel over batch, 128 sequences per core):

* The forward algorithm runs in *scaled probability space*:
      p_{t+1} = (Ê p_t) ⊙ F_t,   Ê = exp(trans - LAMBDA),  F_t = exp(feat_t)
  with per-column scale bookkeeping: periodic column normalizations whose
  reciprocals are Ln'd and summed at the end, plus the deterministic
  LAMBDA*(len+1) correction.  LAMBDA recenters growth so f32 never overflows
  between normalizations.

* Length masking is folded into the *data* (host-side padding, no compute):
  dead steps (t >= len) get a one-hot STOP emission column and
  Ê[STOP,STOP] = 1, which makes dead steps exact no-ops and injects the STOP
  transition exactly once at t = len.  Alive steps get their STOP emission
  killed.  The scan is completely mask-free and exact.

* The gold path score is *also* a CRF scan: restricting emissions to the
  gold path (host masks feats to one-hot columns by tag) makes the same
  recursion compute exp(gold).  It uses unscaled Êg = exp(trans) (gold
  scores are zero-drift) and the same normalization machinery.

* Serial latency is halved by meeting in the middle: forward scans over
  t=0..256 and backward scans over t=512..257 run concurrently, then
  Z = sum_k p_fwd[k] * beta_bwd[k] (and likewise for gold).

* Layout: tags 2-stacked on partitions (group 0 at 0..47, group 1 at
  64..111 - engine APs need 32-aligned bases), batch columns on the free
  dim.  The Z-chain and gold-chain of each direction are fused side by side
  in the free dim ([112, 128] tiles, PSUM slices of one bank), so each tick
  costs one DVE multiply and two small matmuls per direction.

The kernel is self-contained: shapes are hardcoded, host prep is numpy.
"""

import os

import numpy as np

S, B, T = 512, 1024, 48
START, STOP = 46, 47
SP1 = S + 1                  # 513 scan steps (one trailing all-dead step)
N_CORES = 8
BC = B // N_CORES            # 128 sequences per core
G, W = 2, 64                 # 2 tag-groups of 64 batch columns
G1 = 64                      # partition base of group 1
P = G1 + T                   # 112-partition span (rows 48..63 are zero)
W2 = 2 * W                   # fused Z|gold pair width
N_FWD = 257                  # forward ticks: t = 0..256
N_BWD = 256                  # backward ticks: t = 512..257
LAMBDA = 4.875
NEG_PAD = np.float32(-30000.0)
NORM_EVERY = 64
NORM_TICKS = tuple(t for t in range(N_BWD) if (t + 1) % NORM_EVERY == 0)
N_EV = len(NORM_TICKS)       # 4 normalization events per chain
CH = 8                       # feature-DMA chunk size (timesteps)
FBUFS = 12                   # rotating fused-feature chunk buffers

_CACHE = {}


# --------------------------------------------------------------------------
# device program
# --------------------------------------------------------------------------
def _build_nc():
    import concourse.bacc as bacc
    import concourse.mybir as mybir
    from concourse import tile

    f32 = mybir.dt.float32

    nc = bacc.Bacc("TRN2", target_bir_lowering=False, debug=False,
                   enable_asserts=False)

    ft = nc.dram_tensor("ft", [SP1, T, BC], f32, kind="ExternalInput")
    fg = nc.dram_tensor("fg", [SP1, T, BC], f32, kind="ExternalInput")
    trn = nc.dram_tensor("trn", [T, T], f32, kind="ExternalInput")
    lensg = nc.dram_tensor("lensg", [G, W], f32, kind="ExternalInput")
    idn = nc.dram_tensor("idn", [T, T], f32, kind="ExternalInput")
    obd = nc.dram_tensor("obd", [P, G], f32, kind="ExternalInput")
    obc = nc.dram_tensor("obc", [G, P], f32, kind="ExternalInput")
    pinit = nc.dram_tensor("pinit", [P, W2], f32, kind="ExternalInput")
    binit = nc.dram_tensor("binit", [P, W2], f32, kind="ExternalInput")
    ssfix = nc.dram_tensor("ssfix", [T, T], f32, kind="ExternalInput")
    out = nc.dram_tensor("out", [2, BC], f32, kind="ExternalOutput")

    AF = mybir.ActivationFunctionType
    ALU = mybir.AluOpType
    AX = mybir.AxisListType

    gbase = (0, G1)

    # forward chunks cover t=0..255 (32x8) plus t=256 (1); backward chunks
    # cover t=257..512 (32x8), consumed in descending-t order.
    fwd_chunks = [(8 * j, 8) for j in range(32)] + [(256, 1)]
    bwd_chunks = [(257 + 8 * j, 8) for j in range(31, -1, -1)]

    with tile.TileContext(nc) as tc:
        with tc.tile_pool(name="sb", bufs=1) as sb, \
             tc.tile_pool(name="fpool", bufs=FBUFS) as fpool, \
             tc.tile_pool(name="ps", bufs=1, space="PSUM") as ps:

            # ---------------- fused F chunks: DMA + exp ----------------
            # per-t fused layout: [F_t (64 cols) | Fgold_t (64 cols)]
            ftiles = {}

            def emit_chunk(t0, L):
                tl = fpool.tile([P, L * W2], f32, tag="fchunk",
                                name=f"F{t0}")
                nc.gpsimd.memset(tl[32:64, :], 0.0)
                for g in range(G):
                    dst = tl[gbase[g]:gbase[g] + T, 0:L * W2]
                    dst = dst.rearrange("p (t c) -> p t c", c=W2)
                    for h, src_t in ((0, ft), (1, fg)):
                        src = src_t[t0:t0 + L, :, g * W:(g + 1) * W]
                        src = src.rearrange("t i c -> i t c")
                        nc.sync.dma_start(dst[:, :, h * W:(h + 1) * W], src)
                nc.scalar.activation(tl[:, :], tl[:, :], AF.Exp)
                ftiles[t0] = tl

            def f_slice(t):
                if t < 256:
                    t0 = (t // 8) * 8
                elif t == 256:
                    t0 = 256
                else:
                    t0 = 257 + ((t - 257) // 8) * 8
                off = (t - t0) * W2
                return ftiles[t0][:, off:off + W2]

            # ------------- prologue: Ê, stationaries, state -------------
            e0 = sb.tile([T, T], f32)
            nc.sync.dma_start(e0[:, :], trn[:, :])
            ssfix_sb = sb.tile([T, T], f32)
            nc.sync.dma_start(ssfix_sb[:, :], ssfix[:, :])
            idn_sb = sb.tile([T, T], f32)
            nc.sync.dma_start(idn_sb[:, :], idn[:, :])

            biasl = sb.tile([T, 1], f32)
            nc.vector.memset(biasl[:, :], -LAMBDA)
            esb = sb.tile([T, T], f32)     # exp(trans - LAMBDA), [STOP,STOP]=1
            nc.scalar.activation(esb[:, :], e0[:, :], AF.Exp, bias=biasl[:, :])
            nc.vector.tensor_add(esb[:, :], esb[:, :], ssfix_sb[:, :])
            egb = sb.tile([T, T], f32)     # exp(trans), [STOP,STOP]=1
            nc.scalar.activation(egb[:, :], e0[:, :], AF.Exp)
            nc.vector.tensor_add(egb[:, :], egb[:, :], ssfix_sb[:, :])

            et_ps = ps.tile([T, T], f32, tag="misc_ps", name="et_ps")
            nc.tensor.transpose(et_ps[:, :], esb[:, :], idn_sb[:, :])
            est = sb.tile([T, T], f32)
            nc.vector.tensor_copy(est[:, :], et_ps[:, :])
            egt_ps = ps.tile([T, T], f32, tag="misc_ps", name="egt_ps")
            nc.tensor.transpose(egt_ps[:, :], egb[:, :], idn_sb[:, :])
            egt = sb.tile([T, T], f32)
            nc.vector.tensor_copy(egt[:, :], egt_ps[:, :])

            # blockdiag stationaries: wf/wg fwd (transposed blocks),
            # wbk/wgb bwd (untransposed blocks)
            ws = {}
            for nm, blk in (("wf", est), ("wg", egt), ("wbk", esb),
                            ("wgb", egb)):
                wt = sb.tile([P, P], f32, tag=f"w_{nm}", name=f"w_{nm}")
                nc.vector.memset(wt[:, :], 0.0)
                for g in range(G):
                    nc.vector.tensor_copy(
                        wt[gbase[g]:gbase[g] + T, gbase[g]:gbase[g] + T],
                        blk[:, :])
                ws[nm] = wt

            obd_sb = sb.tile([P, G], f32)
            nc.sync.dma_start(obd_sb[:, :], obd[:, :])
            obc_sb = sb.tile([G, P], f32)
            nc.sync.dma_start(obc_sb[:, :], obc[:, :])
            lens_sb = sb.tile([G, W], f32)
            nc.sync.dma_start(lens_sb[:, :], lensg[:, :])

            pf = sb.tile([P, W2], f32)            # [p | r]
            nc.sync.dma_start(pf[:, :], pinit[:, :])
            binit_sb = sb.tile([P, W2], f32)
            nc.sync.dma_start(binit_sb[:, :], binit[:, :])
            bpair = [ps.tile([P, W2], f32, tag="b0", name="b0"),
                     ps.tile([P, W2], f32, tag="b1", name="b1")]
            nc.vector.tensor_copy(bpair[0][:, :], binit_sb[:, :])
            u = sb.tile([P, W2], f32)
            q = [ps.tile([P, W2], f32, tag="q0", name="q0"),
                 ps.tile([P, W2], f32, tag="q1", name="q1")]
            norm_ps = ps.tile([G, W], f32, tag="norm_ps")
            bc_ps = ps.tile([P, W], f32, tag="bc_ps")
            # reciprocal norms: 4 chains x N_EV events x 64 cols
            rn_all = sb.tile([G, 4 * N_EV * W], f32)

            # stage all chunks up-front (pool bufs throttle DMA run-ahead)
            for a, b in zip(fwd_chunks, bwd_chunks + [None]):
                emit_chunk(*a)
                if b is not None:
                    emit_chunk(*b)

            ev_idx = [0, 0, 0, 0]   # per-chain normalization event counter

            def do_norm(state, half, chain):
                e = ev_idx[chain]
                ev_idx[chain] += 1
                sl = state[:, half * W:(half + 1) * W]
                nc.tensor.matmul(norm_ps[:, :], obd_sb[:, :], sl,
                                 start=True, stop=True)
                rn = rn_all[:, (chain * N_EV + e) * W:
                            (chain * N_EV + e + 1) * W]
                nc.vector.reciprocal_approx_fast(rn, norm_ps[:, :])
                nc.tensor.matmul(bc_ps[:, :], obc_sb[:, :], rn,
                                 start=True, stop=True)
                nc.vector.tensor_mul(sl, sl, bc_ps[:, :])

            # ---------------- the scan ----------------
            for tick in range(N_FWD):
                qt = q[tick % 2]
                nc.tensor.matmul(qt[:, 0:W], ws["wf"][:, :], pf[:, 0:W],
                                 start=True, stop=True)
                nc.tensor.matmul(qt[:, W:W2], ws["wg"][:, :], pf[:, W:W2],
                                 start=True, stop=True)
                nc.vector.tensor_mul(pf[:, :], qt[:, :], f_slice(tick))
                if tick in NORM_TICKS:
                    do_norm(pf, 0, 0)
                    do_norm(pf, 1, 1)

                if tick < N_BWD:
                    t = S - tick
                    bcur = bpair[tick % 2]
                    bnxt = bpair[1 - tick % 2]
                    nc.vector.tensor_mul(u[:, :], bcur[:, :], f_slice(t))
                    if tick in NORM_TICKS:
                        do_norm(u, 0, 2)
                        do_norm(u, 1, 3)
                    nc.tensor.matmul(bnxt[:, 0:W], ws["wbk"][:, :], u[:, 0:W],
                                     start=True, stop=True)
                    nc.tensor.matmul(bnxt[:, W:W2], ws["wgb"][:, :],
                                     u[:, W:W2], start=True, stop=True)

            b_fin = bpair[N_BWD % 2]

            # ---------------- combine ----------------
            s = sb.tile([P, W2], f32)
            nc.vector.tensor_mul(s[:, :], pf[:, :], b_fin[:, :])
            zz_ps = ps.tile([G, W2], f32, tag="misc_ps", name="zz_ps")
            nc.tensor.matmul(zz_ps[:, 0:W], obd_sb[:, :], s[:, 0:W],
                             start=True, stop=True)
            nc.tensor.matmul(zz_ps[:, W:W2], obd_sb[:, :], s[:, W:W2],
                             start=True, stop=True)
            zlog = sb.tile([G, W2], f32)
            nc.scalar.activation(zlog[:, :], zz_ps[:, :], AF.Ln)

            lg = sb.tile([G, 4 * N_EV * W], f32)
            nc.scalar.activation(lg[:, :], rn_all[:, :], AF.Ln)
            cc = sb.tile([G, 4 * W], f32)    # -sum_e log rn, per chain
            for chain in range(4):
                src = lg[:, chain * N_EV * W:(chain + 1) * N_EV * W]
                nc.vector.tensor_reduce(
                    cc[:, chain * W:(chain + 1) * W],
                    src.rearrange("p (e c) -> p c e", e=N_EV),
                    axis=AX.X, op=ALU.add, negate=True)

            corr = sb.tile([G, W], f32)
            nc.vector.tensor_scalar(corr[:, :], lens_sb[:, :], 1.0, LAMBDA,
                                    ALU.add, ALU.mult)
            lz = sb.tile([G, W2], f32)       # [logZ | gold]
            nc.vector.tensor_add(lz[:, 0:W], zlog[:, 0:W], cc[:, 0:W])
            nc.vector.tensor_add(lz[:, 0:W], lz[:, 0:W], cc[:, 2 * W:3 * W])
            nc.vector.tensor_add(lz[:, 0:W], lz[:, 0:W], corr[:, :])
            nc.vector.tensor_add(lz[:, W:W2], zlog[:, W:W2], cc[:, W:2 * W])
            nc.vector.tensor_add(lz[:, W:W2], lz[:, W:W2], cc[:, 3 * W:4 * W])
            nc.sync.dma_start(out[0, :].rearrange("(g c) -> g c", g=G),
                              lz[:, 0:W])
            nc.sync.dma_start(out[1, :].rearrange("(g c) -> g c", g=G),
                              lz[:, W:W2])

    nc.compile()
    return nc


# --------------------------------------------------------------------------
# host side
# --------------------------------------------------------------------------
def _host_prep(feats, transitions, tags, lens):
    feats = np.ascontiguousarray(np.asarray(feats, np.float32))
    tags = np.asarray(tags).astype(np.int64)
    lens = np.asarray(lens).astype(np.int64)

    # padded transposed feats [S+1, T, B]
    fT = np.empty((SP1, T, B), np.float32)
    fT[:S] = np.transpose(feats, (0, 2, 1))
    fT[S] = NEG_PAD
    t_idx = np.arange(SP1)[:, None]
    alive = t_idx < lens[None, :]                       # [S+1, B]
    fT[:, STOP, :] = np.where(alive, NEG_PAD, fT[:, STOP, :])
    dead = ~alive
    fT = np.where(dead[:, None, :], NEG_PAD, fT)
    fT[:, STOP, :] = np.where(dead, 0.0, fT[:, STOP, :])

    tags_pad = np.concatenate(
        [np.full((B, 1), START, np.int64), tags,
         np.full((B, 1), STOP, np.int64)], axis=1)      # [B, S+2]
    pos = np.arange(S + 2)[None, :]
    tags_pad = np.where(pos < (lens + 1)[:, None], tags_pad, STOP)

    # gold-masked features: one-hot columns by tag
    m = tags_pad[:, 1:S + 2].T[:, None, :] == np.arange(T)[None, :, None]
    fgold = np.where(m, fT, NEG_PAD).astype(np.float32)  # [S+1, T, B]
    return fT, fgold, lens


def _consts():
    idn = np.eye(T, dtype=np.float32)
    obd = np.zeros((P, G), np.float32)
    obc = np.zeros((G, P), np.float32)
    for g, gb in enumerate((0, G1)):
        obd[gb:gb + T, g] = 1.0
        obc[g, gb:gb + T] = 1.0
    pinit = np.zeros((P, W2), np.float32)
    pinit[START], pinit[G1 + START] = 1.0, 1.0
    binit = np.zeros((P, W2), np.float32)
    binit[STOP], binit[G1 + STOP] = 1.0, 1.0
    ssfix = np.zeros((T, T), np.float32)
    ssfix[STOP, STOP] = 1.0
    return idn, obd, obc, pinit, binit, ssfix


def kernel(feats, transitions, tags, lens):
    from concourse.bass_utils import run_bass_kernel_spmd

    if "nc" not in _CACHE:
        _CACHE["nc"] = _build_nc()
    nc = _CACHE["nc"]

    fT, fgold, lens64 = _host_prep(feats, transitions, tags, lens)
    idn, obd, obc, pinit, binit, ssfix = _consts()
    trans_raw = np.asarray(transitions, np.float32)

    in_maps = []
    for core in range(N_CORES):
        sl = slice(core * BC, (core + 1) * BC)
        in_maps.append({
            "ft": np.ascontiguousarray(fT[:, :, sl]),
            "fg": np.ascontiguousarray(fgold[:, :, sl]),
            "trn": trans_raw,
            "lensg": lens64[sl].astype(np.float32).reshape(G, W),
            "idn": idn,
            "obd": obd,
            "obc": obc,
            "pinit": pinit,
            "binit": binit,
            "ssfix": ssfix,
        })

    res = run_bass_kernel_spmd(
        nc, in_maps, core_ids=list(range(N_CORES)),
        trace=os.environ.get("KERNEL_TRACE", "0") == "1")
    _CACHE["last_result"] = res

    logz = np.concatenate([r["out"][0] for r in res.results])
    gold = np.concatenate([r["out"][1] for r in res.results])
    loss = (logz - gold).mean(dtype=np.float64)
    return np.float32(loss)
